# revision 25
# baseline (speedup 1.0000x reference)
"""GCN layer kernel for 8 Trainium2 NeuronCores.

Math (reference):
    h  = (x @ W1.T + b1) @ W2.T + b2
    A  = adj + I
    r  = rowsum(A) ** -0.5
    out = (r[:,None] * A * r[None,:]) @ h
        = r ⊙ (adj @ (r ⊙ h)) + r ⊙ (r ⊙ h)

Associativity: with h1 = x @ W1.T + b1 and g1 = [r ⊙ h1 | r] (257 cols),
    acc  = adj @ g1 + g1_own            # [rows, 257]  (spmm, K=8192)
    out  = r ⊙ (acc[:, :256] @ W2.T + acc[:, 256] ⊗ b2)

Distribution: row-shard adj and x across 8 cores (1024 rows each).
Per core:
  - stream the 32MB fp32 adj shard via SWDGE cast-DMAs (fp32 HBM -> bf16
    SBUF in one hop; the gpsimd queue carries nothing else so streaming
    starts at t=0);
  - XBAR DMA-transposes (ACT HWDGE ring) land bf16 adjT [P, KB, JT, P]
    resident in SBUF with contiguous per-partition destinations;
  - rowsums as PE matmuls adjT.T @ ones accumulated per row block;
  - mm1 on the local x shard -> h1 (hid on partitions); h1 + rowsum
    AllGathers issue from the *vector* engine queue so they don't block
    any DMA-issuing queue;
  - g1 built by DMA-transposing gathered h1, scaled by r;
  - spmm: per local i-tile 64 bf16 matmuls [128x128]@[128x257] into one
    PSUM bank; small @W2T tail + v*b2 + r_i scale; DMA out.
"""

import sys

if "/opt/trn_rl_repo" not in sys.path:
    sys.path.insert(0, "/opt/trn_rl_repo")

import numpy as np

import concourse.bass as bass
import concourse.mybir as mybir
import concourse.tile as tile
from concourse import bacc
from concourse.bass_utils import run_bass_kernel_spmd

F32 = mybir.dt.float32
BF16 = mybir.dt.bfloat16
P = 128
NCORES = 8


def build_nc(n_nodes=8192, in_f=512, hid=256, out_f=512, cw=2048, debug_dump=False):
    """Build and compile the per-core SPMD kernel."""
    cw = min(cw, n_nodes)
    SH = n_nodes // NCORES      # shard rows per core
    KB = SH // P                # row blocks in shard (= local i tiles)
    JT = n_nodes // P           # global j tiles
    QN = n_nodes // cw          # read chunks per row block
    BPC = cw // P               # 128-blocks per chunk
    FT = in_f // P              # input feature tiles
    HT = hid // P               # hidden tiles
    OF = out_f
    GW = hid + 1                # g1 logical width: [r*h1 | r]
    GWP = hid + 16              # padded stride (32B-aligned tDMA dests)
    IG = min(2, KB)             # i-stripes per mm1 group
    NG = IG * P                 # mm1 rhs free size

    nc = bacc.Bacc(
        "TRN2",
        target_bir_lowering=False,
        debug=False,
        num_devices=NCORES,
        dynamic_dma_scratch_size=4096,
    )
    adj_s = nc.dram_tensor("adj_shard", [SH, n_nodes], F32, kind="ExternalInput")
    x_s = nc.dram_tensor("x_shard", [SH, in_f], F32, kind="ExternalInput")
    W1 = nc.dram_tensor("W1", [hid, in_f], F32, kind="ExternalInput")
    b1 = nc.dram_tensor("b1", [hid], F32, kind="ExternalInput")
    W2 = nc.dram_tensor("W2", [out_f, hid], F32, kind="ExternalInput")
    b2 = nc.dram_tensor("b2", [out_f], F32, kind="ExternalInput")
    out = nc.dram_tensor("out_shard", [SH, OF], F32, kind="ExternalOutput")

    with tile.TileContext(nc) as tc:
        with (
            tc.tile_pool(name="const", bufs=1) as cpool,
            tc.tile_pool(name="big", bufs=1) as bigpool,
            tc.tile_pool(name="adj_in", bufs=2) as adj_in_pool,
            tc.tile_pool(name="chbf", bufs=2) as chbf_pool,
            tc.tile_pool(name="xin", bufs=2) as xin_pool,
            tc.tile_pool(name="xbf", bufs=2) as xbf_pool,
            tc.tile_pool(name="xt", bufs=1) as xt_pool,
            tc.tile_pool(name="acc_bf", bufs=1) as accbf_pool,
            tc.tile_pool(name="vb", bufs=1) as vb_pool,
            tc.tile_pool(name="out_sb", bufs=1) as out_pool,
            tc.tile_pool(name="pmm", bufs=2, space="PSUM") as pmm_pool,
            tc.tile_pool(name="pacc", bufs=2, space="PSUM") as pacc_pool,
            tc.tile_pool(name="prs", bufs=2, space="PSUM") as prs_pool,
            tc.tile_pool(name="dram", bufs=1, space="DRAM") as dram,
        ):
            # ---------- constants ----------
            ones_bf = cpool.tile([1, max(OF, P)], BF16)
            nc.vector.memset(ones_bf, 1.0)
            ones_col = cpool.tile([P, 1], BF16)
            nc.vector.memset(ones_col, 1.0)
            b12_f = cpool.tile([1, hid + out_f], F32)
            nc.sync.dma_start(b12_f[:, :hid], b1.ap()[None, :])
            nc.sync.dma_start(b12_f[:, hid:], b2.ap()[None, :])
            b12_bf = cpool.tile([1, hid + out_f], BF16)
            nc.vector.tensor_copy(b12_bf, b12_f)
            b1_bf = b12_bf[:, :hid]
            b2_bf = b12_bf[:, hid:]
            # b2 broadcast to all partitions (for the v*b2 outer product)
            pb = pmm_pool.tile([P, OF], F32, tag="pmm")
            nc.tensor.matmul(pb, ones_bf[:1, :P], b2_bf[:1, :], start=True, stop=True)
            b2_bcast = cpool.tile([P, OF], BF16)
            nc.scalar.copy(b2_bcast, pb)

            # ---------- adj streaming: cast-DMA + XBAR transpose + PE rowsums ----
            adjT = bigpool.tile([P, KB, JT, P], BF16)
            rowsum_c = cpool.tile([P, KB], F32)
            for k in range(KB):
                for q in range(QN):
                    ch = adj_in_pool.tile([P, cw], F32, tag="adj_in")
                    nc.sync.dma_start(
                        ch, adj_s.ap()[k * P:(k + 1) * P, q * cw:(q + 1) * cw]
                    )
                    chbf = chbf_pool.tile([P, cw], BF16, tag="chbf")
                    if (k * QN + q) % 2 == 0:
                        nc.vector.tensor_copy(chbf, ch)
                    else:
                        nc.scalar.copy(chbf, ch)
                    nc.scalar.dma_start_transpose(
                        adjT[:, k, q * BPC:(q + 1) * BPC, :], chbf
                    )
                prs = prs_pool.tile([P, 1], F32)
                for jt in range(JT):
                    nc.tensor.matmul(
                        prs,
                        adjT[:, k, jt, :],
                        ones_col,
                        start=(jt == 0),
                        stop=(jt == JT - 1),
                    )
                nc.scalar.copy(rowsum_c[:, k:k + 1], prs)

            # ---------- weights: W1T [P, FT, hid], W2T [P, HT, OF] ----------
            W1T = cpool.tile([P, FT, hid], BF16)
            W2T = cpool.tile([P, HT, OF], BF16)
            for s in range(HT):  # W1 row stripes (hid)
                w_in = xin_pool.tile([P, in_f], F32, tag="xin")
                nc.sync.dma_start(w_in, W1.ap()[s * P:(s + 1) * P, :])
                w_bf = xbf_pool.tile([P, in_f], BF16, tag="xbf")
                nc.scalar.copy(w_bf, w_in)
                nc.scalar.dma_start_transpose(
                    W1T[:, :, s * P:(s + 1) * P], w_bf
                )
            for s in range(out_f // P):  # W2 row stripes (out_f)
                w_in = xin_pool.tile([P, in_f], F32, tag="xin")
                nc.sync.dma_start(w_in[:, :hid], W2.ap()[s * P:(s + 1) * P, :])
                w_bf = xbf_pool.tile([P, in_f], BF16, tag="xbf")
                nc.scalar.copy(w_bf[:, :hid], w_in[:, :hid])
                nc.scalar.dma_start_transpose(
                    W2T[:, :, s * P:(s + 1) * P], w_bf[:, :hid]
                )

            # ---------- mm1: h1_c [P, HT, SH] bf16 (hid on partitions) ----------
            h1_c = bigpool.tile([P, HT, SH], BF16)
            for grp in range(SH // NG):
                xt = xt_pool.tile([P, FT, NG], BF16)
                for s in range(IG):
                    row0 = (grp * IG + s) * P
                    x_in = xin_pool.tile([P, in_f], F32, tag="xin")
                    nc.sync.dma_start(x_in, x_s.ap()[row0:row0 + P, :])
                    x_bf = xbf_pool.tile([P, in_f], BF16, tag="xbf")
                    nc.scalar.copy(x_bf, x_in)
                    nc.scalar.dma_start_transpose(
                        xt[:, :, s * P:(s + 1) * P], x_bf
                    )
                for ht in range(HT):
                    pm = pmm_pool.tile([P, OF], F32, tag="pmm")
                    pm1 = pm[:, :NG]
                    nc.tensor.matmul(
                        pm1, b1_bf[:1, ht * P:(ht + 1) * P], ones_bf[:1, :NG],
                        start=True, stop=False,
                    )
                    for ft in range(FT):
                        nc.tensor.matmul(
                            pm1,
                            W1T[:, ft, ht * P:(ht + 1) * P],
                            xt[:, ft, :],
                            start=False, stop=(ft == FT - 1),
                        )
                    nc.scalar.copy(h1_c[:, ht, grp * NG:(grp + 1) * NG], pm1)

            # ---------- AllGather h1 (vector queue; DMA queues stay free) -------
            h1d = dram.tile([HT, P, SH], BF16)
            for ht in range(HT):
                nc.sync.dma_start(h1d[ht], h1_c[:, ht, :])
            h1g = dram.tile([NCORES * HT, P, SH], BF16, addr_space="Shared")
            nc.gpsimd.collective_compute(
                "AllGather",
                mybir.AluOpType.bypass,
                replica_groups=[list(range(NCORES))],
                ins=[h1d.opt()],
                outs=[h1g.opt()],
            )

            # ---------- AllGather rowsums; r vectors ----------
            rsd = dram.tile([P, KB], F32)
            nc.sync.dma_start(rsd, rowsum_c)
            rsg = dram.tile([NCORES * P, KB], F32, addr_space="Shared")
            nc.gpsimd.collective_compute(
                "AllGather",
                mybir.AluOpType.bypass,
                replica_groups=[list(range(NCORES))],
                ins=[rsd.opt()],
                outs=[rsg.opt()],
            )
            # r_sb [P, NCORES, KB]: r for global tile jt = (c, k) at [:, c, k]
            rs_t = cpool.tile([P, NCORES, KB], F32)
            nc.sync.dma_start(rs_t, rsg.rearrange("(c p) k -> p c k", p=P))
            r_sb = cpool.tile([P, NCORES, KB], F32)
            nc.vector.tensor_scalar_add(rs_t, rs_t, 1.0)
            nc.vector.reciprocal(rs_t, rs_t)
            nc.scalar.sqrt(r_sb, rs_t)
            # local r for own rows
            r_own = cpool.tile([P, KB], F32)
            ro_t = cpool.tile([P, KB], F32)
            nc.vector.tensor_scalar_add(ro_t, rowsum_c, 1.0)
            nc.vector.reciprocal(ro_t, ro_t)
            nc.scalar.sqrt(r_own, ro_t)

            # ---------- g1 [P, JT, GWP] = [r ⊙ h1ᵀ | r] for all rows ----------
            g1 = bigpool.tile([P, JT, GWP], BF16)
            for c in range(NCORES):
                for ht in range(HT):
                    # dest[p, k, q] = h1g[c*HT+ht, q, k*P+p]
                    nc.scalar.dma_start_transpose(
                        g1[:, c * KB:(c + 1) * KB, ht * P:(ht + 1) * P],
                        h1g[c * HT + ht, :, :],
                    )
            for jt in range(JT):
                c, k = jt // KB, jt % KB
                rc = r_sb[:, c, k:k + 1]
                nc.vector.tensor_scalar_mul(g1[:, jt, :hid], g1[:, jt, :hid], rc)
                nc.vector.tensor_copy(g1[:, jt, hid:GW], rc)
            # own-row g1 from local h1 (identity term)
            g1own = bigpool.tile([P, KB, GWP], BF16)
            for ht in range(HT):
                nc.scalar.dma_start_transpose(
                    g1own[:, :, ht * P:(ht + 1) * P], h1_c[:, ht, :]
                )
            for k in range(KB):
                rc = r_own[:, k:k + 1]
                nc.vector.tensor_scalar_mul(g1own[:, k, :hid], g1own[:, k, :hid], rc)
                nc.vector.tensor_copy(g1own[:, k, hid:GW], rc)

            # ---------- debug dumps ----------
            if debug_dump:
                d_adjT = nc.dram_tensor(
                    "d_adjT", [P, KB, JT, P], BF16, kind="ExternalOutput"
                )
                nc.sync.dma_start(d_adjT.ap(), adjT)
                d_g1 = nc.dram_tensor("d_g1", [P, JT, GWP], BF16, kind="ExternalOutput")
                nc.sync.dma_start(d_g1.ap(), g1)
                d_h1 = nc.dram_tensor("d_h1", [P, HT, SH], BF16, kind="ExternalOutput")
                nc.sync.dma_start(d_h1.ap(), h1_c)
                d_rs = nc.dram_tensor("d_rs", [P, KB], F32, kind="ExternalOutput")
                nc.sync.dma_start(d_rs.ap(), rowsum_c)

            # ---------- spmm + tail ----------
            for it in range(KB):
                acc = pacc_pool.tile([P, GW], F32)
                for jt in range(JT):
                    nc.tensor.matmul(
                        acc,
                        adjT[:, it, jt, :],
                        g1[:, jt, :GW],
                        start=(jt == 0),
                        stop=(jt == JT - 1),
                    )
                # acc += g1_own (identity term), cast to bf16 for the tail
                nc.vector.tensor_tensor(
                    acc, acc, g1own[:, it, :GW], mybir.AluOpType.add
                )
                accbf = accbf_pool.tile([P, hid], BF16)
                nc.scalar.copy(accbf, acc[:, :hid])
                vcol = accbf_pool.tile([P, 1], F32, tag="vcol")
                nc.vector.tensor_copy(vcol, acc[:, hid:GW])
                # accT [P(h1), HT, P(i)] via SBUF->SBUF tDMA
                accT = accbf_pool.tile([P, HT, P], BF16, tag="accT")
                nc.scalar.dma_start_transpose(accT, accbf)
                # tail: out = r_own ⊙ (accT.T @ W2T + v ⊗ b2)
                pt = pmm_pool.tile([P, OF], F32, tag="pmm")
                for ht in range(HT):
                    nc.tensor.matmul(
                        pt, accT[:, ht, :], W2T[:, ht, :],
                        start=(ht == 0), stop=(ht == HT - 1),
                    )
                vb = vb_pool.tile([P, OF], BF16)
                nc.vector.tensor_scalar_mul(vb, b2_bcast, vcol)
                nc.vector.tensor_tensor(pt, pt, vb, mybir.AluOpType.add)
                o_sb = out_pool.tile([P, OF], F32)
                nc.vector.tensor_scalar_mul(o_sb, pt, r_own[:, it:it + 1])
                nc.sync.dma_start(out.ap()[it * P:(it + 1) * P, :], o_sb)

    nc.compile()
    return nc


_NC_CACHE = {}


def _get_nc(key=8192):
    if key not in _NC_CACHE:
        _NC_CACHE[key] = build_nc(n_nodes=key)
    return _NC_CACHE[key]


def kernel(x, adj, W1, b1, W2, b2):
    """Full-input entry point: shard internally across 8 cores, return full output."""
    n = adj.shape[0]
    nc = _get_nc(n)
    sh = n // NCORES
    x = np.ascontiguousarray(np.asarray(x, dtype=np.float32))
    adj = np.ascontiguousarray(np.asarray(adj, dtype=np.float32))
    W1 = np.ascontiguousarray(np.asarray(W1, dtype=np.float32))
    b1 = np.ascontiguousarray(np.asarray(b1, dtype=np.float32))
    W2 = np.ascontiguousarray(np.asarray(W2, dtype=np.float32))
    b2 = np.ascontiguousarray(np.asarray(b2, dtype=np.float32))
    in_maps = [
        {
            "adj_shard": adj[c * sh:(c + 1) * sh],
            "x_shard": x[c * sh:(c + 1) * sh],
            "W1": W1,
            "b1": b1,
            "W2": W2,
            "b2": b2,
        }
        for c in range(NCORES)
    ]
    res = run_bass_kernel_spmd(nc, in_maps, list(range(NCORES)), trace=False)
    return np.concatenate(
        [res.results[c]["out_shard"] for c in range(NCORES)], axis=0
    )
